# revision 5
# baseline (speedup 1.0000x reference)
"""Trainium2 Bass kernel for nn_GAT_87617332838818.

Mathematical collapse: the reference GAT aggregates ``alpha * hp[:, dst]``
over incoming edges per destination node.  Since the softmax weights alpha
sum to exactly 1 within each destination segment and the aggregated message
``hp[dst]`` is constant within the segment, the whole message-passing step
is the identity: ``out[n] = hp[n]``.  The network therefore reduces to a
per-node 3-layer MLP:

    logits = W2r @ elu(W1r @ elu(W0r @ x^T))        (per node column)

with W0r = W0.reshape(96,128), W1r = W1.reshape(96,96), W2r = W2.reshape(40,96)
(head-concat order matches the plain reshape).  Verified numerically against
the reference: rel fro err 4e-7 (f32), 1.1e-3 with the fp16 device pipeline.

Device strategy (8 NeuronCores, node-sharded 6250 rows each):
  - activations kept feature-on-partition: xT [128, n], h [96, n]
  - ELU decomposed as  elu(p) + 1 = max(p,0) + min(exp(p),1)
    The "+1" inflation is folded into the next layer's bias
    (c = W_next @ ones, applied inside the exp / relu passes), and the
    final layer's bias is subtracted in the output drain pass.
  - per mid layer: ACT does e=exp(p+nb) (PSUM->SBUF fp16),
    DVE does r=max(p+nb,0), DVE/GPSIMD does h=min(e,1)+r (scalar_tensor_tensor)
  - output: DVE tensor_scalar subtract (PSUM->SBUF fp16) then DMA out.
"""

import os
import sys

import numpy as np

for _p in ("/root/.axon_site/_ro/trn_rl_repo", "/opt/trn_rl_repo"):
    if os.path.isdir(_p) and _p not in sys.path:
        sys.path.append(_p)

import concourse.bass as bass
import concourse.tile as tile
from concourse import bacc, mybir
from concourse.bass_utils import run_bass_kernel_spmd

N_CORES = 8
N_PER = 6250            # 50000 / 8
D_IN = 128
D_HID = 96
D_OUT = 40
GROUP = 1024            # elementwise/psum group size (2 PSUM banks)
MM_N = 512              # matmul moving free-dim chunk (1 PSUM bank)

F16 = mybir.dt.float16
F32 = mybir.dt.float32

# stt engine per mid layer: True -> gpsimd, False -> vector (load balance knob)
STT_ON_GPSIMD = (False, False)

_groups = [GROUP] * (N_PER // GROUP)
if N_PER % GROUP:
    _groups.append(N_PER % GROUP)


def _build_program() -> bass.Bass:
    nc = bacc.Bacc(None, target_bir_lowering=False, debug=False)

    xT = nc.declare_dram_parameter("xT", [D_IN, N_PER], F16, isOutput=False)
    w0t = nc.declare_dram_parameter("w0t", [D_IN, D_HID], F16, isOutput=False)
    w1t = nc.declare_dram_parameter("w1t", [D_HID, D_HID], F16, isOutput=False)
    w2t = nc.declare_dram_parameter("w2t", [D_HID, D_OUT], F16, isOutput=False)
    nb0 = nc.declare_dram_parameter("nb0", [D_HID, 1], F32, isOutput=False)
    nb1 = nc.declare_dram_parameter("nb1", [D_HID, 1], F32, isOutput=False)
    cb2 = nc.declare_dram_parameter("cb2", [D_OUT, 1], F32, isOutput=False)
    yT = nc.declare_dram_parameter("yT", [D_OUT, N_PER], F16, isOutput=True)

    with tile.TileContext(nc) as tc:
        with (
            tc.tile_pool(name="consts", bufs=1) as consts,
            tc.tile_pool(name="xin", bufs=3) as xpool,
            tc.tile_pool(name="act", bufs=3) as apool,
            tc.tile_pool(name="hid", bufs=3) as hpool,
            tc.tile_pool(name="outp", bufs=3) as opool,
            tc.tile_pool(name="psA", bufs=2, space="PSUM") as psA,
            tc.tile_pool(name="psB", bufs=2, space="PSUM") as psB,
        ):
            w0_sb = consts.tile([D_IN, D_HID], F16, tag="w0")
            w1_sb = consts.tile([D_HID, D_HID], F16, tag="w1")
            w2_sb = consts.tile([D_HID, D_OUT], F16, tag="w2")
            nb0_sb = consts.tile([D_HID, 1], F32, tag="nb0")
            nb1_sb = consts.tile([D_HID, 1], F32, tag="nb1")
            cb2_sb = consts.tile([D_OUT, 1], F32, tag="cb2")
            nc.sync.dma_start(w0_sb[:], w0t[:])
            nc.sync.dma_start(w1_sb[:], w1t[:])
            nc.sync.dma_start(w2_sb[:], w2t[:])
            nc.sync.dma_start(nb0_sb[:], nb0[:])
            nc.sync.dma_start(nb1_sb[:], nb1[:])
            nc.sync.dma_start(cb2_sb[:], cb2[:])

            col = 0
            for fd in _groups:
                sl = slice(col, col + fd)
                col += fd

                xt = xpool.tile([D_IN, GROUP], F16, tag="xt")
                nc.sync.dma_start(xt[:, :fd], xT[:, sl])

                # ---- layer 0: p0 = W0r @ x ----
                p0 = psA.tile([D_HID, GROUP], F32, tag="A")
                for j0 in range(0, fd, MM_N):
                    j1 = min(j0 + MM_N, fd)
                    nc.tensor.matmul(p0[:, j0:j1], w0_sb[:], xt[:, j0:j1],
                                     start=True, stop=True)
                h1 = _elu_plus1(nc, tc, apool, hpool, p0, nb0_sb, fd,
                                STT_ON_GPSIMD[0], "h1")

                # ---- layer 1: p1 = W1r @ (h1+1), true preact = p1 - c1 ----
                p1 = psB.tile([D_HID, GROUP], F32, tag="B")
                for j0 in range(0, fd, MM_N):
                    j1 = min(j0 + MM_N, fd)
                    nc.tensor.matmul(p1[:, j0:j1], w1_sb[:], h1[:, j0:j1],
                                     start=True, stop=True)
                h2 = _elu_plus1(nc, tc, apool, hpool, p1, nb1_sb, fd,
                                STT_ON_GPSIMD[1], "h2")

                # ---- layer 2: logits = W2r @ (h2+1) - c2 ----
                p2 = psA.tile([D_OUT, GROUP], F32, tag="A")
                for j0 in range(0, fd, MM_N):
                    j1 = min(j0 + MM_N, fd)
                    nc.tensor.matmul(p2[:, j0:j1], w2_sb[:], h2[:, j0:j1],
                                     start=True, stop=True)
                o = opool.tile([D_OUT, GROUP], F16, tag="o")
                nc.vector.tensor_scalar_sub(o[:, :fd], p2[:, :fd], cb2_sb[:])
                nc.sync.dma_start(yT[:, sl], o[:, :fd])

    nc.compile()
    return nc


def _elu_plus1(nc, tc, apool, hpool, p, nb_sb, fd, stt_on_gpsimd, htag):
    """h = elu(p + nb) + 1 = max(p+nb, 0) + min(exp(p+nb), 1), fp16 in SBUF."""
    e = apool.tile([D_HID, GROUP], F16, tag="e")
    nc.scalar.activation(e[:, :fd], p[:, :fd],
                         mybir.ActivationFunctionType.Exp, bias=nb_sb[:])
    r = apool.tile([D_HID, GROUP], F16, tag="r")
    nc.vector.tensor_scalar(r[:, :fd], p[:, :fd], nb_sb[:], 0.0,
                            mybir.AluOpType.add, mybir.AluOpType.max)
    h = hpool.tile([D_HID, GROUP], F16, tag=htag)
    eng = nc.gpsimd if stt_on_gpsimd else nc.vector
    eng.scalar_tensor_tensor(h[:, :fd], e[:, :fd], 1.0, r[:, :fd],
                             mybir.AluOpType.min, mybir.AluOpType.add)
    return h


_prog_cache = []
last_result = None


def kernel(**inputs) -> np.ndarray:
    global last_result
    x = np.asarray(inputs["x"], np.float32)           # [50000, 128]
    W0 = np.asarray(inputs["W0"], np.float32).reshape(D_HID, D_IN)
    W1 = np.asarray(inputs["W1"], np.float32).reshape(D_HID, D_HID)
    W2 = np.asarray(inputs["W2"], np.float32).reshape(D_OUT, D_HID)

    n = x.shape[0]
    assert n == N_CORES * N_PER, f"unexpected node count {n}"

    xT16 = np.ascontiguousarray(x.T.astype(np.float16))     # [128, 50000]
    w0t = np.ascontiguousarray(W0.T.astype(np.float16))     # [128, 96]
    w1t = np.ascontiguousarray(W1.T.astype(np.float16))     # [96, 96]
    w2t = np.ascontiguousarray(W2.T.astype(np.float16))     # [96, 40]
    nb0 = np.zeros((D_HID, 1), np.float32)
    nb1 = -W1.sum(axis=1, keepdims=True).astype(np.float32)  # -(W1 @ ones)
    cb2 = W2.sum(axis=1, keepdims=True).astype(np.float32)   # +(W2 @ ones)

    if not _prog_cache:
        _prog_cache.append(_build_program())
    nc = _prog_cache[0]

    in_maps = [
        dict(
            xT=np.ascontiguousarray(xT16[:, i * N_PER:(i + 1) * N_PER]),
            w0t=w0t, w1t=w1t, w2t=w2t, nb0=nb0, nb1=nb1, cb2=cb2,
        )
        for i in range(N_CORES)
    ]
    res = run_bass_kernel_spmd(nc, in_maps, list(range(N_CORES)))
    last_result = res
    out = np.concatenate(
        [np.asarray(res.results[i]["yT"], np.float32).T for i in range(N_CORES)],
        axis=0,
    )
    return out


if __name__ == "__main__":
    data = np.load("/tmp/gat_inputs.npz")
    y = kernel(**{k: data[k] for k in data.files})
    print("out", y.shape, y.dtype, "absmax", np.abs(y).max())
